# revision 9
# baseline (speedup 1.0000x reference)
"""Trainium2 Bass kernel for GQA attention layer (B=1, S=2048, H=4096,
32 Q heads / 8 KV heads, head_dim 128, RoPE with arbitrary tables).

Sharding: tensor-parallel over heads across 8 NeuronCores — core c gets
Q heads 4c..4c+3 and KV head c (Wq/Wk/Wv column shards, Wo row shard).
Each core computes its partial o_proj output [2048, 4096]; the host sums
the 8 partials (equivalent of the all-reduce).

Per-core compute (all matmuls bf16 with fp32 PSUM accumulation):
  Phase A: qT/kT/vT = W.T @ hs.T in [d, s] layout (N=512 matmuls),
           RoPE applied via rotate-half partition swap (SBUF-to-SBUF DMA)
           + elementwise DVE ops; v transposed to [s, d] chunks on PE.
  Phase B: flash-style causal attention per (head, q-range, k-tile):
           scoresT[k,q] = kT.T @ qT, probsT = exp(scale*scores) (masked
           on diagonal tiles), attn_oT[d,q] += v[k,d].T @ probsT,
           denom[1,q] += ones.T @ probsT; normalize via reciprocal +
           gpsimd partition_broadcast + DVE multiply. No max-subtraction
           (scores are bounded; fp32 exp is exact enough).
  Phase C: partial o_proj [s, hidden] = attn_oT.T @ Wo_shard.
"""

import sys
from contextlib import ExitStack

sys.path.insert(0, "/opt/trn_rl_repo")

import numpy as np
import ml_dtypes

import concourse.bass as bass
import concourse.bacc as bacc
import concourse.mybir as mybir
import concourse.tile as tile
from concourse.bass_utils import run_bass_kernel_spmd
from concourse.masks import make_identity

BF16 = mybir.dt.bfloat16
F32 = mybir.dt.float32

N_CORES = 8
S = 2048
HID = 4096
D = 128
NQ = 4  # q heads per core
KC = HID // 128  # 32 hidden-dim chunks
NQR = S // 512  # 4 q ranges of 512
NST = S // 128  # 16 s-tiles of 128
NHO = HID // 512  # 8 output column tiles of 512
SCALE = 1.0 / float(np.sqrt(D))

_CACHE: dict = {}


def _build_nc():
    nc = bacc.Bacc(None, target_bir_lowering=False, debug=False)

    hst_d = nc.dram_tensor("hst", [NQR, 128, KC, 512], BF16, kind="ExternalInput")
    wq_d = nc.dram_tensor("wq", [128, KC, NQ * D], BF16, kind="ExternalInput")
    wk_d = nc.dram_tensor("wk", [128, KC, D], BF16, kind="ExternalInput")
    wv_d = nc.dram_tensor("wv", [128, KC, D], BF16, kind="ExternalInput")
    wo_d = nc.dram_tensor("wo", [128, NQ, HID], BF16, kind="ExternalInput")
    cos_d = nc.dram_tensor("cos2", [128, S], F32, kind="ExternalInput")
    sin_d = nc.dram_tensor("sin2", [128, S], F32, kind="ExternalInput")
    out_d = nc.dram_tensor("out", [S, HID], F32, kind="ExternalOutput")

    with tile.TileContext(nc) as tc, ExitStack() as stack:
        # ---- constants (live whole kernel) ----
        const = stack.enter_context(tc.tile_pool(name="const", bufs=1))
        identity = const.tile([128, 128], BF16)
        make_identity(nc, identity[:])
        ones = const.tile([128, 1], BF16)
        nc.gpsimd.memset(ones[:], 1.0)
        cos_sb = const.tile([128, S], F32)
        sin_sb = const.tile([128, S], F32)
        nc.sync.dma_start(cos_sb[:], cos_d[:])
        nc.sync.dma_start(sin_sb[:], sin_d[:])
        # causal masks for the 4 diagonal-block positions: tile rows are k
        # (128), cols are q (512); block at position p keeps q >= k + 128p.
        masks = []
        for p in range(4):
            m = const.tile([128, 512], BF16, tag=f"mask{p}")
            nc.gpsimd.memset(m[:], 1.0)
            nc.gpsimd.affine_select(
                out=m[:],
                in_=m[:],
                pattern=[[1, 512]],
                compare_op=mybir.AluOpType.is_ge,
                fill=0.0,
                base=-128 * p,
                channel_multiplier=-1,
            )
            masks.append(m)

        # ---- persistent activations ----
        act = stack.enter_context(tc.tile_pool(name="act", bufs=1))
        qt_sb = [
            act.tile([128, S], BF16, tag=f"qt{h}", name=f"qt{h}") for h in range(NQ)
        ]
        kt_sb = act.tile([128, S], BF16, tag="kt")
        vt_sb = act.tile([128, S], BF16, tag="vt")
        v_sb = act.tile([128, NST, 128], BF16, tag="v")  # [d->p? no: k-chunks]
        attn_sb = [
            act.tile([128, S], BF16, tag=f"attn{h}", name=f"attn{h}")
            for h in range(NQ)
        ]

        # ================= Phase A: QKV projections + RoPE =================
        with (
            tc.tile_pool(name="wqkv", bufs=1) as wqkv,
            tc.tile_pool(name="hstp", bufs=2) as hstp,
            tc.tile_pool(name="rope", bufs=3) as rope,
            tc.tile_pool(name="psA", bufs=3, space="PSUM") as psA,
            tc.tile_pool(name="psT", bufs=2, space="PSUM") as psT,
        ):
            wq_sb = wqkv.tile([128, KC, NQ * D], BF16)
            wk_sb = wqkv.tile([128, KC, D], BF16)
            wv_sb = wqkv.tile([128, KC, D], BF16)
            nc.sync.dma_start(wq_sb[:], wq_d[:])
            nc.sync.dma_start(wk_sb[:], wk_d[:])
            nc.sync.dma_start(wv_sb[:], wv_d[:])

            def rope_evict(ps, dst_tile, qr):
                """dst[0:64]  = x0*cos - x1*sin
                dst[64:128] = x1*cos + x0*sin   (x0=ps[0:64], x1=ps[64:128])"""
                sl = slice(qr * 512, (qr + 1) * 512)
                raw = rope.tile([128, 512], F32, tag="raw")
                nc.vector.tensor_copy(raw[:], ps[:])
                swp = rope.tile([128, 512], F32, tag="swp")
                nc.sync.dma_start(swp[0:64, :], raw[64:128, :])
                nc.sync.dma_start(swp[64:128, :], raw[0:64, :])
                t1 = rope.tile([128, 512], F32, tag="t1")
                t2 = rope.tile([128, 512], F32, tag="t2")
                nc.vector.tensor_mul(t1[:], raw[:], cos_sb[:, sl])
                nc.vector.tensor_mul(t2[:], swp[:], sin_sb[:, sl])
                nc.vector.tensor_sub(dst_tile[0:64, sl], t1[0:64, :], t2[0:64, :])
                nc.vector.tensor_add(
                    dst_tile[64:128, sl], t1[64:128, :], t2[64:128, :]
                )

            for qr in range(NQR):
                hst_t = hstp.tile([128, KC, 512], BF16)
                nc.sync.dma_start(hst_t[:], hst_d[qr])
                # 6 outputs: 4 q heads, k, v — all [128, 512] for this qr
                jobs = [("q", h) for h in range(NQ)] + [("k", 0), ("v", 0)]
                for kind, h in jobs:
                    ps = psA.tile([128, 512], F32)
                    for c in range(KC):
                        if kind == "q":
                            lhsT = wq_sb[:, c, h * D : (h + 1) * D]
                        elif kind == "k":
                            lhsT = wk_sb[:, c, :]
                        else:
                            lhsT = wv_sb[:, c, :]
                        nc.tensor.matmul(
                            ps[:],
                            lhsT,
                            hst_t[:, c, :],
                            start=(c == 0),
                            stop=(c == KC - 1),
                        )
                    if kind == "q":
                        rope_evict(ps, qt_sb[h], qr)
                    elif kind == "k":
                        rope_evict(ps, kt_sb, qr)
                    else:
                        sl = slice(qr * 512, (qr + 1) * 512)
                        nc.vector.tensor_copy(vt_sb[:, sl], ps[:])
                # transpose this qr's v slice into [k, d] chunks
                for kt in range(qr * 4, qr * 4 + 4):
                    pst = psT.tile([128, 128], BF16)
                    nc.tensor.transpose(
                        pst[:], vt_sb[:, kt * 128 : (kt + 1) * 128], identity[:]
                    )
                    nc.vector.tensor_copy(v_sb[:, kt, :], pst[:])

        # ================= Phase B: causal attention =================
        wo_pool = stack.enter_context(tc.tile_pool(name="wo", bufs=1))
        wo_sb = wo_pool.tile([128, NQ, HID], BF16)
        nc.sync.dma_start(wo_sb[:], wo_d[:])

        with (
            tc.tile_pool(name="probs", bufs=3) as probs_p,
            tc.tile_pool(name="expb", bufs=2) as expb_p,
            tc.tile_pool(name="den", bufs=2) as den_p,
            tc.tile_pool(name="bcast", bufs=2) as bcast_p,
            tc.tile_pool(name="psS", bufs=3, space="PSUM") as psS,
            tc.tile_pool(name="psO", bufs=2, space="PSUM") as psO,
            tc.tile_pool(name="psD", bufs=2, space="PSUM") as psD,
        ):
            for qr in range(NQR):
                n_kt = 4 * (qr + 1)
                qsl = slice(qr * 512, (qr + 1) * 512)
                for h in range(NQ):
                    ps_o = psO.tile([128, 512], F32)
                    ps_d = psD.tile([1, 512], F32)
                    s_tiles = {}

                    def mm_scores(kt):
                        ps_s = psS.tile([128, 512], F32, tag="s")
                        nc.tensor.matmul(
                            ps_s[:],
                            kt_sb[:, kt * 128 : (kt + 1) * 128],
                            qt_sb[h][:, qsl],
                            start=True,
                            stop=True,
                        )
                        s_tiles[kt] = ps_s

                    mm_scores(0)
                    for kt in range(n_kt):
                        if kt + 1 < n_kt:
                            mm_scores(kt + 1)
                        ps_s = s_tiles.pop(kt)
                        pt = probs_p.tile([128, 512], BF16, tag="pt")
                        p_idx = kt - 4 * qr
                        if p_idx >= 0:
                            et = expb_p.tile([128, 512], BF16, tag="et")
                            nc.scalar.activation(
                                et[:],
                                ps_s[:],
                                mybir.ActivationFunctionType.Exp,
                                scale=SCALE,
                            )
                            nc.vector.tensor_mul(pt[:], et[:], masks[p_idx][:])
                        else:
                            nc.scalar.activation(
                                pt[:],
                                ps_s[:],
                                mybir.ActivationFunctionType.Exp,
                                scale=SCALE,
                            )
                        nc.tensor.matmul(
                            ps_o[:],
                            v_sb[:, kt, :],
                            pt[:],
                            start=(kt == 0),
                            stop=(kt == n_kt - 1),
                            skip_group_check=True,
                        )
                        nc.tensor.matmul(
                            ps_d[:],
                            ones[:],
                            pt[:],
                            start=(kt == 0),
                            stop=(kt == n_kt - 1),
                            skip_group_check=True,
                        )
                    recip = den_p.tile([1, 512], F32, tag="recip")
                    nc.vector.reciprocal(recip[:], ps_d[:])
                    bc = bcast_p.tile([128, 512], F32, tag="bc")
                    nc.gpsimd.partition_broadcast(bc[:], recip[:])
                    nc.vector.tensor_mul(attn_sb[h][:, qsl], ps_o[:], bc[:])

        # ================= Phase C: partial o_proj =================
        with (
            tc.tile_pool(name="ostage", bufs=4) as ostage,
            tc.tile_pool(name="psC", bufs=8, space="PSUM") as psC,
        ):
            for st in range(NST):
                ssl = slice(st * 128, (st + 1) * 128)
                for hg in range(2):
                    ps_c = [
                        psC.tile([128, 512], F32, tag="c", name=f"c{st}_{hg}_{j}")
                        for j in range(4)
                    ]
                    for h in range(NQ):
                        lhsT = attn_sb[h][:, ssl]
                        for j in range(4):
                            ho = hg * 4 + j
                            nc.tensor.matmul(
                                ps_c[j][:],
                                lhsT,
                                wo_sb[:, h, ho * 512 : (ho + 1) * 512],
                                start=(h == 0),
                                stop=(h == NQ - 1),
                                skip_group_check=True,
                            )
                    for j in range(4):
                        ho = hg * 4 + j
                        stg = ostage.tile([128, 512], F32, tag="stg")
                        nc.vector.tensor_copy(stg[:], ps_c[j][:])
                        nc.sync.dma_start(
                            out_d[ssl, ho * 512 : (ho + 1) * 512], stg[:]
                        )

    nc.compile()
    return nc


def _get_nc():
    if "nc" not in _CACHE:
        _CACHE["nc"] = _build_nc()
    return _CACHE["nc"]


def _bf16(x):
    return np.ascontiguousarray(x.astype(ml_dtypes.bfloat16))


def _prep_in_maps(hidden_states, sin_table, cos_table, Wq, Wk, Wv, Wo):
    hs0 = np.asarray(hidden_states, np.float32).reshape(S, HID)
    # hst[qr, p, c, s] = hs0[qr*512 + s, c*128 + p]
    hst = _bf16(hs0.reshape(NQR, 512, KC, 128).transpose(0, 3, 2, 1))
    cosT = np.asarray(cos_table, np.float32).T  # [64, S]
    sinT = np.asarray(sin_table, np.float32).T
    cos2 = np.ascontiguousarray(np.concatenate([cosT, cosT], 0))  # [128, S]
    sin2 = np.ascontiguousarray(np.concatenate([sinT, sinT], 0))
    Wq = np.asarray(Wq, np.float32)
    Wk = np.asarray(Wk, np.float32)
    Wv = np.asarray(Wv, np.float32)
    Wo = np.asarray(Wo, np.float32)

    in_maps = []
    for c in range(N_CORES):
        wq_c = Wq[:, c * 512 : (c + 1) * 512]  # 4 q heads
        wk_c = Wk[:, c * 128 : (c + 1) * 128]  # 1 kv head
        wv_c = Wv[:, c * 128 : (c + 1) * 128]
        wo_c = Wo[c * 512 : (c + 1) * 512, :]  # matching rows
        in_maps.append(
            {
                "hst": hst,
                "wq": _bf16(wq_c.reshape(KC, 128, NQ * D).swapaxes(0, 1)),
                "wk": _bf16(wk_c.reshape(KC, 128, D).swapaxes(0, 1)),
                "wv": _bf16(wv_c.reshape(KC, 128, D).swapaxes(0, 1)),
                "wo": _bf16(wo_c.reshape(NQ, 128, HID).swapaxes(0, 1)),
                "cos2": cos2,
                "sin2": sin2,
            }
        )
    return in_maps


def run(trace=False, **inputs):
    nc = _get_nc()
    in_maps = _prep_in_maps(**inputs)
    res = run_bass_kernel_spmd(
        nc, in_maps, core_ids=list(range(N_CORES)), trace=trace
    )
    partials = np.stack([res.results[c]["out"] for c in range(N_CORES)])
    out = partials.sum(axis=0, dtype=np.float32).reshape(1, S, HID)
    return out, res


def kernel(**inputs):
    out, _ = run(trace=False, **inputs)
    return out


# revision 10
# speedup vs baseline: 1.1412x; 1.1412x over previous
"""Trainium2 Bass kernel for GQA attention layer (B=1, S=2048, H=4096,
32 Q heads / 8 KV heads, head_dim 128, RoPE with arbitrary tables).

Sharding: tensor-parallel over heads across 8 NeuronCores — core c gets
Q heads 4c..4c+3 and KV head c (Wq/Wk/Wv column shards, Wo row shard).
Each core computes its partial o_proj output [2048, 4096]; the host sums
the 8 partials (equivalent of the all-reduce).

Per-core compute (all matmuls bf16 with fp32 PSUM accumulation):
  Phase A: qT/kT/vT = W.T @ hs.T in [d, s] layout (N=512 matmuls),
           RoPE applied via rotate-half partition swap (SBUF-to-SBUF DMA)
           + elementwise DVE ops; v transposed to [s, d] chunks on PE.
  Phase B: flash-style causal attention per (head, q-range, k-tile):
           scoresT[k,q] = kT.T @ qT, probsT = exp(scale*scores),
           attn_oT[d,q] += v[k,d].T @ probsT, denom[1,q] += ones.T @
           probsT; diagonal k-tiles narrowed to the unmasked column range
           with a single triangular 128-col mask multiply. Normalization
           via fast reciprocal + gpsimd partition_broadcast + DVE mul.
           No max-subtraction (scores are bounded; fp32 exp is exact
           enough).
  Phase C: partial o_proj [s, hidden] = attn_oT.T @ Wo_shard.
"""

import sys
from contextlib import ExitStack

sys.path.insert(0, "/opt/trn_rl_repo")

import numpy as np
import ml_dtypes

import concourse.bass as bass
import concourse.bacc as bacc
import concourse.mybir as mybir
import concourse.tile as tile
from concourse.bass_utils import run_bass_kernel_spmd
from concourse.masks import make_identity

BF16 = mybir.dt.bfloat16
F32 = mybir.dt.float32

N_CORES = 8
S = 2048
HID = 4096
D = 128
NQ = 4  # q heads per core
KC = HID // 128  # 32 hidden-dim chunks
NQR = S // 512  # 4 q ranges of 512
NST = S // 128  # 16 s-tiles of 128
NHO = HID // 512  # 8 output column tiles of 512
SCALE = 1.0 / float(np.sqrt(D))

_CACHE: dict = {}


def _build_nc():
    nc = bacc.Bacc(None, target_bir_lowering=False, debug=False)

    hst_d = nc.dram_tensor("hst", [NQR, 128, KC, 512], BF16, kind="ExternalInput")
    wq_d = nc.dram_tensor("wq", [NQ, 128, KC, D], BF16, kind="ExternalInput")
    wk_d = nc.dram_tensor("wk", [128, KC, D], BF16, kind="ExternalInput")
    wv_d = nc.dram_tensor("wv", [128, KC, D], BF16, kind="ExternalInput")
    wo_d = nc.dram_tensor("wo", [128, NQ, HID], BF16, kind="ExternalInput")
    cos_d = nc.dram_tensor("cos2", [128, S], F32, kind="ExternalInput")
    sin_d = nc.dram_tensor("sin2", [128, S], F32, kind="ExternalInput")
    out_d = nc.dram_tensor("out", [S, HID], F32, kind="ExternalOutput")

    with tile.TileContext(nc) as tc, ExitStack() as stack:
        # ---- pools that live the whole kernel ----
        const = stack.enter_context(tc.tile_pool(name="const", bufs=1))
        act = stack.enter_context(tc.tile_pool(name="act", bufs=1))
        qt_sb = [
            act.tile([128, S], BF16, tag=f"qt{h}", name=f"qt{h}") for h in range(NQ)
        ]
        kt_sb = act.tile([128, S], BF16, tag="kt")
        vt_sb = act.tile([128, S], BF16, tag="vt")
        v_sb = act.tile([128, NST, 128], BF16, tag="v")  # [s,d] chunks per k-tile
        attn_sb = [
            act.tile([128, S], BF16, tag=f"attn{h}", name=f"attn{h}")
            for h in range(NQ)
        ]

        # ================= Phase A: QKV projections + RoPE =================
        with (
            tc.tile_pool(name="wqkv", bufs=1) as wqkv,
            tc.tile_pool(name="hstp", bufs=2) as hstp,
            tc.tile_pool(name="rope", bufs=3) as rope,
            tc.tile_pool(name="psA", bufs=3, space="PSUM") as psA,
            tc.tile_pool(name="psT", bufs=2, space="PSUM") as psT,
        ):
            # DMA order matters at startup: get hst[0] + wk + rope tables in
            # first so the k-projection (first job) can start ASAP.
            hst_tiles = []
            hst_t0 = hstp.tile([128, KC, 512], BF16, tag="hst", name="hst0")
            for r in range(4):
                nc.sync.dma_start(
                    hst_t0[:, r * 8 : (r + 1) * 8, :],
                    hst_d[0, :, r * 8 : (r + 1) * 8, :],
                )
            hst_tiles.append(hst_t0)
            wk_sb = wqkv.tile([128, KC, D], BF16)
            nc.sync.dma_start(wk_sb[:], wk_d[:])
            cos_sb = const.tile([128, S], F32)
            sin_sb = const.tile([128, S], F32)
            nc.sync.dma_start(cos_sb[:], cos_d[:])
            nc.sync.dma_start(sin_sb[:], sin_d[:])
            wv_sb = wqkv.tile([128, KC, D], BF16)
            nc.sync.dma_start(wv_sb[:], wv_d[:])
            wq_sb = [
                wqkv.tile([128, KC, D], BF16, tag=f"wq{h}", name=f"wq{h}")
                for h in range(NQ)
            ]
            for h in range(NQ):
                nc.sync.dma_start(wq_sb[h][:], wq_d[h])

            identity = const.tile([128, 128], BF16)
            make_identity(nc, identity[:])
            ones = const.tile([128, 1], BF16)
            nc.gpsimd.memset(ones[:], 1.0)
            # triangular mask for the diagonal 128x128 subtile: rows are k,
            # cols are q; keep q >= k.
            tri = const.tile([128, 128], BF16)
            nc.gpsimd.memset(tri[:], 1.0)
            nc.gpsimd.affine_select(
                out=tri[:],
                in_=tri[:],
                pattern=[[1, 128]],
                compare_op=mybir.AluOpType.is_ge,
                fill=0.0,
                base=0,
                channel_multiplier=-1,
            )

            def rope_evict(ps, dst_tile, qr):
                """dst[0:64]  = x0*cos - x1*sin
                dst[64:128] = x1*cos + x0*sin   (x0=ps[0:64], x1=ps[64:128])"""
                sl = slice(qr * 512, (qr + 1) * 512)
                raw = rope.tile([128, 512], F32, tag="raw")
                nc.vector.tensor_copy(raw[:], ps[:])
                swp = rope.tile([128, 512], F32, tag="swp")
                nc.sync.dma_start(swp[0:64, :], raw[64:128, :])
                nc.sync.dma_start(swp[64:128, :], raw[0:64, :])
                t1 = rope.tile([128, 512], F32, tag="t1")
                t2 = rope.tile([128, 512], F32, tag="t2")
                nc.vector.tensor_mul(t1[:], raw[:], cos_sb[:, sl])
                nc.vector.tensor_mul(t2[:], swp[:], sin_sb[:, sl])
                nc.vector.tensor_sub(dst_tile[0:64, sl], t1[0:64, :], t2[0:64, :])
                nc.vector.tensor_add(
                    dst_tile[64:128, sl], t1[64:128, :], t2[64:128, :]
                )

            for qr in range(NQR):
                if qr + 1 < NQR:
                    nxt = hstp.tile([128, KC, 512], BF16, tag="hst", name=f"hst{qr+1}")
                    nc.sync.dma_start(nxt[:], hst_d[qr + 1])
                    hst_tiles.append(nxt)
                hst_t = hst_tiles[qr]
                # k and v first (their weights arrive first)
                jobs = [("k", 0), ("v", 0)] + [("q", h) for h in range(NQ)]
                for kind, h in jobs:
                    ps = psA.tile([128, 512], F32)
                    for c in range(KC):
                        if kind == "q":
                            lhsT = wq_sb[h][:, c, :]
                        elif kind == "k":
                            lhsT = wk_sb[:, c, :]
                        else:
                            lhsT = wv_sb[:, c, :]
                        nc.tensor.matmul(
                            ps[:],
                            lhsT,
                            hst_t[:, c, :],
                            start=(c == 0),
                            stop=(c == KC - 1),
                        )
                    if kind == "q":
                        rope_evict(ps, qt_sb[h], qr)
                    elif kind == "k":
                        rope_evict(ps, kt_sb, qr)
                    else:
                        sl = slice(qr * 512, (qr + 1) * 512)
                        nc.vector.tensor_copy(vt_sb[:, sl], ps[:])
                # transpose this qr's v slice into [s, d] chunks
                for kt in range(qr * 4, qr * 4 + 4):
                    pst = psT.tile([128, 128], BF16)
                    nc.tensor.transpose(
                        pst[:], vt_sb[:, kt * 128 : (kt + 1) * 128], identity[:]
                    )
                    nc.vector.tensor_copy(v_sb[:, kt, :], pst[:])

        # ================= Phase B: causal attention =================
        wo_pool = stack.enter_context(tc.tile_pool(name="wo", bufs=1))
        wo_sb = wo_pool.tile([128, NQ, HID], BF16)
        nc.sync.dma_start(wo_sb[:], wo_d[:])

        with (
            tc.tile_pool(name="probs", bufs=3) as probs_p,
            tc.tile_pool(name="den", bufs=2) as den_p,
            tc.tile_pool(name="bcast", bufs=2) as bcast_p,
            tc.tile_pool(name="psS", bufs=3, space="PSUM") as psS,
            tc.tile_pool(name="psO", bufs=3, space="PSUM") as psO,
            tc.tile_pool(name="psD", bufs=2, space="PSUM") as psD,
        ):
            for qr in range(NQR):
                n_kt = 4 * (qr + 1)
                qsl = slice(qr * 512, (qr + 1) * 512)
                for h in range(NQ):
                    ps_o = psO.tile([128, 512], F32, tag="o", name=f"o{qr}_{h}")
                    ps_d = psD.tile([1, 512], F32, tag="d", name=f"d{qr}_{h}")
                    s_tiles = {}

                    def mm_scores(kt):
                        # diagonal tiles: only columns q >= 128*p are unmasked
                        p_idx = kt - 4 * qr
                        c0 = 128 * p_idx if p_idx > 0 else 0
                        ps_s = psS.tile(
                            [128, 512], F32, tag="s", name=f"s{qr}_{h}_{kt}"
                        )
                        nc.tensor.matmul(
                            ps_s[:, c0:512],
                            kt_sb[:, kt * 128 : (kt + 1) * 128],
                            qt_sb[h][:, qr * 512 + c0 : (qr + 1) * 512],
                            start=True,
                            stop=True,
                        )
                        s_tiles[kt] = (ps_s, c0)

                    mm_scores(0)
                    for kt in range(n_kt):
                        if kt + 1 < n_kt:
                            mm_scores(kt + 1)
                        ps_s, c0 = s_tiles.pop(kt)
                        p_idx = kt - 4 * qr
                        pt = probs_p.tile(
                            [128, 512], BF16, tag="pt", name=f"pt{qr}_{h}_{kt}"
                        )
                        nc.scalar.activation(
                            pt[:, c0:512],
                            ps_s[:, c0:512],
                            mybir.ActivationFunctionType.Exp,
                            scale=SCALE,
                        )
                        if p_idx >= 0:
                            # triangular mask on the diagonal 128-col subtile
                            nc.vector.tensor_mul(
                                pt[:, c0 : c0 + 128],
                                pt[:, c0 : c0 + 128],
                                tri[:],
                            )
                        nc.tensor.matmul(
                            ps_o[:, c0:512],
                            v_sb[:, kt, :],
                            pt[:, c0:512],
                            start=(kt == 0),
                            stop=(kt == n_kt - 1),
                            skip_group_check=True,
                        )
                        nc.tensor.matmul(
                            ps_d[:, c0:512],
                            ones[:],
                            pt[:, c0:512],
                            start=(kt == 0),
                            stop=(kt == n_kt - 1),
                            skip_group_check=True,
                        )
                    recip = den_p.tile([1, 512], F32, tag="recip")
                    nc.vector.reciprocal_approx_fast(out=recip[:], in_=ps_d[:])
                    bc = bcast_p.tile([128, 512], F32, tag="bc")
                    nc.gpsimd.partition_broadcast(bc[:], recip[:])
                    nc.vector.tensor_mul(attn_sb[h][:, qsl], ps_o[:], bc[:])

        # ================= Phase C: partial o_proj =================
        with (
            tc.tile_pool(name="ostage", bufs=6) as ostage,
            tc.tile_pool(name="psC", bufs=8, space="PSUM") as psC,
        ):
            for st in range(NST):
                ssl = slice(st * 128, (st + 1) * 128)
                for hg in range(2):
                    ps_c = [
                        psC.tile([128, 512], F32, tag="c", name=f"c{st}_{hg}_{j}")
                        for j in range(4)
                    ]
                    for h in range(NQ):
                        lhsT = attn_sb[h][:, ssl]
                        for j in range(4):
                            ho = hg * 4 + j
                            nc.tensor.matmul(
                                ps_c[j][:],
                                lhsT,
                                wo_sb[:, h, ho * 512 : (ho + 1) * 512],
                                start=(h == 0),
                                stop=(h == NQ - 1),
                                skip_group_check=True,
                            )
                    for j in range(4):
                        ho = hg * 4 + j
                        stg = ostage.tile([128, 512], F32, tag="stg")
                        # alternate evict engines: DVE and ACT are both idle-ish
                        if j % 2 == 0:
                            nc.vector.tensor_copy(stg[:], ps_c[j][:])
                        else:
                            nc.scalar.copy(stg[:], ps_c[j][:])
                        nc.sync.dma_start(
                            out_d[ssl, ho * 512 : (ho + 1) * 512], stg[:]
                        )

    nc.compile()
    return nc


def _get_nc():
    if "nc" not in _CACHE:
        _CACHE["nc"] = _build_nc()
    return _CACHE["nc"]


def _bf16(x):
    return np.ascontiguousarray(x.astype(ml_dtypes.bfloat16))


def _prep_in_maps(hidden_states, sin_table, cos_table, Wq, Wk, Wv, Wo):
    hs0 = np.asarray(hidden_states, np.float32).reshape(S, HID)
    # hst[qr, p, c, s] = hs0[qr*512 + s, c*128 + p]
    hst = _bf16(hs0.reshape(NQR, 512, KC, 128).transpose(0, 3, 2, 1))
    cosT = np.asarray(cos_table, np.float32).T  # [64, S]
    sinT = np.asarray(sin_table, np.float32).T
    cos2 = np.ascontiguousarray(np.concatenate([cosT, cosT], 0))  # [128, S]
    sin2 = np.ascontiguousarray(np.concatenate([sinT, sinT], 0))
    Wq = np.asarray(Wq, np.float32)
    Wk = np.asarray(Wk, np.float32)
    Wv = np.asarray(Wv, np.float32)
    Wo = np.asarray(Wo, np.float32)

    in_maps = []
    for c in range(N_CORES):
        wq_c = Wq[:, c * 512 : (c + 1) * 512]  # 4 q heads
        wk_c = Wk[:, c * 128 : (c + 1) * 128]  # 1 kv head
        wv_c = Wv[:, c * 128 : (c + 1) * 128]
        wo_c = Wo[c * 512 : (c + 1) * 512, :]  # matching rows
        # wq per-head-major: [h, p, c, d] with element Wq_c[c*128+p, h*128+d]
        wq_l = wq_c.reshape(KC, 128, NQ, D).transpose(2, 1, 0, 3)
        in_maps.append(
            {
                "hst": hst,
                "wq": _bf16(wq_l),
                "wk": _bf16(wk_c.reshape(KC, 128, D).swapaxes(0, 1)),
                "wv": _bf16(wv_c.reshape(KC, 128, D).swapaxes(0, 1)),
                "wo": _bf16(wo_c.reshape(NQ, 128, HID).swapaxes(0, 1)),
                "cos2": cos2,
                "sin2": sin2,
            }
        )
    return in_maps


def run(trace=False, **inputs):
    nc = _get_nc()
    in_maps = _prep_in_maps(**inputs)
    res = run_bass_kernel_spmd(
        nc, in_maps, core_ids=list(range(N_CORES)), trace=trace
    )
    partials = np.stack([res.results[c]["out"] for c in range(N_CORES)])
    out = partials.sum(axis=0, dtype=np.float32).reshape(1, S, HID)
    return out, res


def kernel(**inputs):
    out, _ = run(trace=False, **inputs)
    return out


# revision 14
# speedup vs baseline: 1.1787x; 1.0328x over previous
"""Trainium2 Bass kernel for GQA attention layer (B=1, S=2048, H=4096,
32 Q heads / 8 KV heads, head_dim 128, RoPE with arbitrary tables).

Sharding: tensor-parallel over heads across 8 NeuronCores — core c gets
Q heads 4c..4c+3 and KV head c (Wq/Wk/Wv column shards, Wo row shard).
Each core computes its partial o_proj output [2048, 4096]; the host sums
the 8 partials (equivalent of the all-reduce).

Per-core compute (all matmuls bf16 with fp32 PSUM accumulation):
  Phase A: qT/kT/vT = W.T @ hs.T in [d, s] layout (N=512 matmuls),
           RoPE applied via rotate-half partition swap (SBUF-to-SBUF DMA)
           + elementwise DVE ops; v transposed to [s, d] chunks on PE.
  Phase B: flash-style causal attention per (head, q-range, k-tile):
           scoresT[k,q] = kT.T @ qT, probsT = exp(scale*scores),
           attn_oT[d,q] += v[k,d].T @ probsT, denom[1,q] += ones.T @
           probsT; diagonal k-tiles narrowed to the unmasked column range
           with a single triangular 128-col mask multiply. Normalization
           via fast reciprocal + gpsimd partition_broadcast + DVE mul.
           No max-subtraction (scores are bounded; fp32 exp is exact
           enough).
  Phase C: partial o_proj [s, hidden] = attn_oT.T @ Wo_shard.
"""

import sys
from contextlib import ExitStack

sys.path.insert(0, "/opt/trn_rl_repo")

import numpy as np
import ml_dtypes

import concourse.bass as bass
import concourse.bacc as bacc
import concourse.mybir as mybir
import concourse.tile as tile
from concourse.bass_utils import run_bass_kernel_spmd
from concourse.masks import make_identity

BF16 = mybir.dt.bfloat16
F32 = mybir.dt.float32

N_CORES = 8
S = 2048
HID = 4096
D = 128
NQ = 4  # q heads per core
KC = HID // 128  # 32 hidden-dim chunks
NQR = S // 512  # 4 q ranges of 512
NST = S // 128  # 16 s-tiles of 128
NHO = HID // 512  # 8 output column tiles of 512
SCALE = 1.0 / float(np.sqrt(D))

_CACHE: dict = {}


def _build_nc():
    nc = bacc.Bacc(None, target_bir_lowering=False, debug=False)

    hst_d = nc.dram_tensor("hst", [NQR, 128, KC, 512], BF16, kind="ExternalInput")
    wq_d = nc.dram_tensor("wq", [NQ, 128, KC, D], BF16, kind="ExternalInput")
    wk_d = nc.dram_tensor("wk", [128, KC, D], BF16, kind="ExternalInput")
    wv_d = nc.dram_tensor("wv", [128, KC, D], BF16, kind="ExternalInput")
    wo_d = nc.dram_tensor("wo", [128, NQ, HID], BF16, kind="ExternalInput")
    cos_d = nc.dram_tensor("cos2", [128, S], F32, kind="ExternalInput")
    sin_d = nc.dram_tensor("sin2", [128, S], F32, kind="ExternalInput")
    out_d = nc.dram_tensor("out", [S, HID], F32, kind="ExternalOutput")

    with tile.TileContext(nc) as tc, ExitStack() as stack:
        # ---- pools that live the whole kernel ----
        const = stack.enter_context(tc.tile_pool(name="const", bufs=1))
        act = stack.enter_context(tc.tile_pool(name="act", bufs=1))
        qt_sb = [
            act.tile([128, S], BF16, tag=f"qt{h}", name=f"qt{h}") for h in range(NQ)
        ]
        kt_sb = act.tile([128, S], BF16, tag="kt")
        vt_sb = act.tile([128, S], BF16, tag="vt")
        v_sb = act.tile([128, NST, 128], BF16, tag="v")  # [s,d] chunks per k-tile
        attn_sb = [
            act.tile([128, S], BF16, tag=f"attn{h}", name=f"attn{h}")
            for h in range(NQ)
        ]
        # B-phase SBUF pools are allocated up-front (NOT from space reused
        # from the A-phase pools) — otherwise the first B eviction picks up
        # a WAR dependency on the tail of phase A and the PE/DVE/GPSIMD
        # engines form a multi-microsecond circular stall.
        probs_p = stack.enter_context(tc.tile_pool(name="probs", bufs=3))
        den_p = stack.enter_context(tc.tile_pool(name="den", bufs=2))
        bcast_p = stack.enter_context(tc.tile_pool(name="bcast", bufs=2))

        # ================= Phase A: QKV projections + RoPE =================
        with (
            tc.tile_pool(name="wqkv", bufs=1) as wqkv,
            tc.tile_pool(name="hstp", bufs=2) as hstp,
            tc.tile_pool(name="rope", bufs=2) as rope,
            tc.tile_pool(name="psA", bufs=3, space="PSUM") as psA,
            tc.tile_pool(name="psT", bufs=2, space="PSUM") as psT,
        ):
            # DMA order matters at startup: get hst[0] + wk + rope tables in
            # first so the k-projection (first job) can start ASAP.
            hst_tiles = []
            hst_t0 = hstp.tile([128, KC, 512], BF16, tag="hst", name="hst0")
            for r in range(4):
                nc.sync.dma_start(
                    hst_t0[:, r * 8 : (r + 1) * 8, :],
                    hst_d[0, :, r * 8 : (r + 1) * 8, :],
                )
            hst_tiles.append(hst_t0)
            wk_sb = wqkv.tile([128, KC, D], BF16)
            nc.sync.dma_start(wk_sb[:], wk_d[:])
            cos_sb = const.tile([128, S], F32)
            sin_sb = const.tile([128, S], F32)
            nc.sync.dma_start(cos_sb[:], cos_d[:])
            nc.sync.dma_start(sin_sb[:], sin_d[:])
            wv_sb = wqkv.tile([128, KC, D], BF16)
            nc.sync.dma_start(wv_sb[:], wv_d[:])
            wq_sb = [
                wqkv.tile([128, KC, D], BF16, tag=f"wq{h}", name=f"wq{h}")
                for h in range(NQ)
            ]
            for h in range(NQ):
                nc.sync.dma_start(wq_sb[h][:], wq_d[h])

            identity = const.tile([128, 128], BF16)
            make_identity(nc, identity[:])
            ones = const.tile([128, 1], BF16)
            nc.gpsimd.memset(ones[:], 1.0)
            # triangular mask for the diagonal 128x128 subtile: rows are k,
            # cols are q; keep q >= k.
            tri = const.tile([128, 128], BF16)
            nc.gpsimd.memset(tri[:], 1.0)
            nc.gpsimd.affine_select(
                out=tri[:],
                in_=tri[:],
                pattern=[[1, 128]],
                compare_op=mybir.AluOpType.is_ge,
                fill=0.0,
                base=0,
                channel_multiplier=-1,
            )

            def rope_evict(ps, dst_tile, qr):
                """dst[0:64]  = x0*cos - x1*sin
                dst[64:128] = x1*cos + x0*sin   (x0=ps[0:64], x1=ps[64:128])"""
                sl = slice(qr * 512, (qr + 1) * 512)
                raw = rope.tile([128, 512], F32, tag="raw")
                nc.vector.tensor_copy(raw[:], ps[:])
                swp = rope.tile([128, 512], F32, tag="swp")
                nc.sync.dma_start(swp[0:64, :], raw[64:128, :])
                nc.sync.dma_start(swp[64:128, :], raw[0:64, :])
                # in-place: raw *= cos, swp *= sin
                nc.vector.tensor_mul(raw[:], raw[:], cos_sb[:, sl])
                nc.vector.tensor_mul(swp[:], swp[:], sin_sb[:, sl])
                nc.vector.tensor_sub(dst_tile[0:64, sl], raw[0:64, :], swp[0:64, :])
                nc.vector.tensor_add(
                    dst_tile[64:128, sl], raw[64:128, :], swp[64:128, :]
                )

            for qr in range(NQR):
                if qr + 1 < NQR:
                    nxt = hstp.tile([128, KC, 512], BF16, tag="hst", name=f"hst{qr+1}")
                    nc.sync.dma_start(nxt[:], hst_d[qr + 1])
                    hst_tiles.append(nxt)
                hst_t = hst_tiles[qr]
                # k and v first (their weights arrive first)
                jobs = [("k", 0), ("v", 0)] + [("q", h) for h in range(NQ)]
                for kind, h in jobs:
                    ps = psA.tile([128, 512], F32)
                    for c in range(KC):
                        if kind == "q":
                            lhsT = wq_sb[h][:, c, :]
                        elif kind == "k":
                            lhsT = wk_sb[:, c, :]
                        else:
                            lhsT = wv_sb[:, c, :]
                        nc.tensor.matmul(
                            ps[:],
                            lhsT,
                            hst_t[:, c, :],
                            start=(c == 0),
                            stop=(c == KC - 1),
                        )
                    if kind == "q":
                        rope_evict(ps, qt_sb[h], qr)
                    elif kind == "k":
                        rope_evict(ps, kt_sb, qr)
                    else:
                        sl = slice(qr * 512, (qr + 1) * 512)
                        nc.vector.tensor_copy(vt_sb[:, sl], ps[:])
                # transpose this qr's v slice into [s, d] chunks
                for kt in range(qr * 4, qr * 4 + 4):
                    pst = psT.tile([128, 128], BF16)
                    nc.tensor.transpose(
                        pst[:], vt_sb[:, kt * 128 : (kt + 1) * 128], identity[:]
                    )
                    nc.vector.tensor_copy(v_sb[:, kt, :], pst[:])

        # ================= Phase B: causal attention =================
        wo_pool = stack.enter_context(tc.tile_pool(name="wo", bufs=1))
        wo_sb = wo_pool.tile([128, NQ, HID], BF16)
        nc.sync.dma_start(wo_sb[:], wo_d[:])

        with (
            tc.tile_pool(name="psS", bufs=3, space="PSUM") as psS,
            tc.tile_pool(name="psO", bufs=3, space="PSUM") as psO,
            tc.tile_pool(name="psD", bufs=2, space="PSUM") as psD,
        ):
            for qr in range(NQR):
                n_kt = 4 * (qr + 1)
                qsl = slice(qr * 512, (qr + 1) * 512)
                for h in range(NQ):
                    ps_o = psO.tile([128, 512], F32, tag="o", name=f"o{qr}_{h}")
                    ps_d = psD.tile([1, 512], F32, tag="d", name=f"d{qr}_{h}")
                    s_tiles = {}

                    def mm_scores(kt):
                        # diagonal tiles: only columns q >= 128*p are unmasked
                        p_idx = kt - 4 * qr
                        c0 = 128 * p_idx if p_idx > 0 else 0
                        ps_s = psS.tile(
                            [128, 512], F32, tag="s", name=f"s{qr}_{h}_{kt}"
                        )
                        nc.tensor.matmul(
                            ps_s[:, c0:512],
                            kt_sb[:, kt * 128 : (kt + 1) * 128],
                            qt_sb[h][:, qr * 512 + c0 : (qr + 1) * 512],
                            start=True,
                            stop=True,
                        )
                        s_tiles[kt] = (ps_s, c0)

                    # 2-deep scores lookahead: scores(kt+2) is issued to the
                    # PE before MM_o(kt) so the ACT exp latency for tile kt
                    # is fully hidden behind PE work.
                    mm_scores(0)
                    if n_kt > 1:
                        mm_scores(1)
                    for kt in range(n_kt):
                        if kt + 2 < n_kt:
                            mm_scores(kt + 2)
                        ps_s, c0 = s_tiles.pop(kt)
                        p_idx = kt - 4 * qr
                        pt = probs_p.tile(
                            [128, 512], BF16, tag="pt", name=f"pt{qr}_{h}_{kt}"
                        )
                        nc.scalar.activation(
                            pt[:, c0:512],
                            ps_s[:, c0:512],
                            mybir.ActivationFunctionType.Exp,
                            scale=SCALE,
                        )
                        if p_idx >= 0:
                            # triangular mask on the diagonal 128-col subtile
                            nc.vector.tensor_mul(
                                pt[:, c0 : c0 + 128],
                                pt[:, c0 : c0 + 128],
                                tri[:],
                            )
                        nc.tensor.matmul(
                            ps_o[:, c0:512],
                            v_sb[:, kt, :],
                            pt[:, c0:512],
                            start=(kt == 0),
                            stop=(kt == n_kt - 1),
                            skip_group_check=True,
                        )
                        nc.tensor.matmul(
                            ps_d[:, c0:512],
                            ones[:],
                            pt[:, c0:512],
                            start=(kt == 0),
                            stop=(kt == n_kt - 1),
                            skip_group_check=True,
                        )
                    recip = den_p.tile([1, 512], F32, tag="recip")
                    nc.vector.reciprocal_approx_fast(out=recip[:], in_=ps_d[:])
                    bc = bcast_p.tile([128, 512], F32, tag="bc")
                    nc.gpsimd.partition_broadcast(bc[:], recip[:])
                    nc.vector.tensor_mul(attn_sb[h][:, qsl], ps_o[:], bc[:])

        # ================= Phase C: partial o_proj =================
        with (
            tc.tile_pool(name="ostage", bufs=6) as ostage,
            tc.tile_pool(name="psC", bufs=8, space="PSUM") as psC,
        ):
            for st in range(NST):
                ssl = slice(st * 128, (st + 1) * 128)
                for hg in range(2):
                    ps_c = [
                        psC.tile([128, 512], F32, tag="c", name=f"c{st}_{hg}_{j}")
                        for j in range(4)
                    ]
                    for h in range(NQ):
                        lhsT = attn_sb[h][:, ssl]
                        for j in range(4):
                            ho = hg * 4 + j
                            nc.tensor.matmul(
                                ps_c[j][:],
                                lhsT,
                                wo_sb[:, h, ho * 512 : (ho + 1) * 512],
                                start=(h == 0),
                                stop=(h == NQ - 1),
                                skip_group_check=True,
                            )
                    for j in range(4):
                        ho = hg * 4 + j
                        stg = ostage.tile([128, 512], F32, tag="stg")
                        # alternate evict engines: DVE and ACT are both idle-ish
                        if j % 2 == 0:
                            nc.vector.tensor_copy(stg[:], ps_c[j][:])
                        else:
                            nc.scalar.copy(stg[:], ps_c[j][:])
                        nc.sync.dma_start(
                            out_d[ssl, ho * 512 : (ho + 1) * 512], stg[:]
                        )

    nc.compile()
    return nc


def _get_nc():
    if "nc" not in _CACHE:
        _CACHE["nc"] = _build_nc()
    return _CACHE["nc"]


def _bf16(x):
    return np.ascontiguousarray(x.astype(ml_dtypes.bfloat16))


def _prep_in_maps(hidden_states, sin_table, cos_table, Wq, Wk, Wv, Wo):
    hs0 = np.asarray(hidden_states, np.float32).reshape(S, HID)
    # hst[qr, p, c, s] = hs0[qr*512 + s, c*128 + p]
    hst = _bf16(hs0.reshape(NQR, 512, KC, 128).transpose(0, 3, 2, 1))
    cosT = np.asarray(cos_table, np.float32).T  # [64, S]
    sinT = np.asarray(sin_table, np.float32).T
    cos2 = np.ascontiguousarray(np.concatenate([cosT, cosT], 0))  # [128, S]
    sin2 = np.ascontiguousarray(np.concatenate([sinT, sinT], 0))
    Wq = np.asarray(Wq, np.float32)
    Wk = np.asarray(Wk, np.float32)
    Wv = np.asarray(Wv, np.float32)
    Wo = np.asarray(Wo, np.float32)

    in_maps = []
    for c in range(N_CORES):
        wq_c = Wq[:, c * 512 : (c + 1) * 512]  # 4 q heads
        wk_c = Wk[:, c * 128 : (c + 1) * 128]  # 1 kv head
        wv_c = Wv[:, c * 128 : (c + 1) * 128]
        wo_c = Wo[c * 512 : (c + 1) * 512, :]  # matching rows
        # wq per-head-major: [h, p, c, d] with element Wq_c[c*128+p, h*128+d]
        wq_l = wq_c.reshape(KC, 128, NQ, D).transpose(2, 1, 0, 3)
        in_maps.append(
            {
                "hst": hst,
                "wq": _bf16(wq_l),
                "wk": _bf16(wk_c.reshape(KC, 128, D).swapaxes(0, 1)),
                "wv": _bf16(wv_c.reshape(KC, 128, D).swapaxes(0, 1)),
                "wo": _bf16(wo_c.reshape(NQ, 128, HID).swapaxes(0, 1)),
                "cos2": cos2,
                "sin2": sin2,
            }
        )
    return in_maps


def run(trace=False, **inputs):
    nc = _get_nc()
    in_maps = _prep_in_maps(**inputs)
    res = run_bass_kernel_spmd(
        nc, in_maps, core_ids=list(range(N_CORES)), trace=trace
    )
    partials = np.stack([res.results[c]["out"] for c in range(N_CORES)])
    out = partials.sum(axis=0, dtype=np.float32).reshape(1, S, HID)
    return out, res


def kernel(**inputs):
    out, _ = run(trace=False, **inputs)
    return out
